# revision 1
# baseline (speedup 1.0000x reference)
"""BinaryLinear (65536x1024 @ binarized 1024x1024) on 8 TRN2 NeuronCores.

out = x @ (sign(w) * mean(|w|, axis=1)).T

Strategy (data-parallel per sharding hint):
  - shard x along tokens: 8192 rows per core; replicate w.
  - per core, once: compute alpha = mean|w| per output row (ACT Abs with
    fused accumulate), w_bin = sign(w)*alpha via a single DVE bitwise
    copysign, then PE-transpose w_bin into a resident SBUF tile
    ST = w_bin.T stored as float32r (e8m11) -- exact for the sign,
    2^-12 rounding on alpha.
  - per 128-token tile: round x to float32r on ACT (exact e8m11 RNE), PE-
    transpose the 8 [128t x 128i] blocks at the f32r rate (1.5 cyc/row)
    packed 4-per-PSUM-bank so one DVE copy drains 4 transposes, then 16
    accumulating float32r matmuls (1 cyc/row, fp32 PSUM accumulation into a
    single 2-bank tile) against ST, one ACT copy PSUM->SBUF, DMA out in
    natural [t, o] layout. The PE stream is software-pipelined (tile tt+1's
    transposes emitted before tile tt's matmuls -> zero steady-state PE gaps
    in the cost-model timeline), and loads/stores use separate HWDGE rings
    (loads on nc.sync/SP, stores on nc.scalar/ACT) so they never FIFO-block
    each other.

float32r matmul runs at 1 cycle/row (vs 4 for fp32) with e8m11 operand
rounding; with sign-weights the products are near-exact, giving ~1.8e-4
relative error vs the fp32 reference. HW-measured (paired reps contrast):
~280 us per full pass per core (best pairs ~258 us), vs a 259 us PE floor
(218 us matmul streaming + 41 us transposes) and ~198 us memory roofline;
cost-model timeline: 259 us/pass + ~38 us one-time DMA-bound setup.
"""

import sys

for _p in ("/opt/trn_rl_repo", "/root/.axon_site/_ro/trn_rl_repo"):
    if _p not in sys.path:
        sys.path.insert(0, _p)

import numpy as np

import concourse.mybir as mybir
import concourse.tile as tile
from concourse import bacc
from concourse.masks import make_identity

TOKENS, IN_F, OUT_F = 65536, 1024, 1024
N_CORES = 8
T_PER_CORE = TOKENS // N_CORES  # 8192
P = 128
T_TILES = T_PER_CORE // P  # 64
KT = IN_F // P  # 8 contraction tiles
NFREE = 512  # PSUM bank free dim (fp32)
NT = OUT_F // NFREE  # 2

F32 = mybir.dt.float32
F32R = mybir.dt.float32r


def build_nc(reps: int = 1):
    nc = bacc.Bacc()
    x = nc.declare_dram_parameter("x", [T_PER_CORE, IN_F], F32, isOutput=False)
    w = nc.declare_dram_parameter("w", [OUT_F, IN_F], F32, isOutput=False)
    out = nc.declare_dram_parameter("out", [T_PER_CORE, OUT_F], F32, isOutput=True)

    with tile.TileContext(nc) as tc:
        with (
            tc.tile_pool(name="const", bufs=1) as cpool,
            tc.tile_pool(name="st", bufs=1) as stpool,
            tc.tile_pool(name="wproc", bufs=2) as wpool,
            tc.tile_pool(name="xin", bufs=4) as xpool,
            tc.tile_pool(name="xt", bufs=4) as xtpool,
            tc.tile_pool(name="outp", bufs=3) as opool,
            tc.tile_pool(name="ptp", bufs=4, space="PSUM") as ptp_pool,
            tc.tile_pool(name="pmm", bufs=2, space="PSUM") as pmm_pool,
        ):
            ident = cpool.tile([P, P], F32)
            make_identity(nc, ident[:])
            identr = cpool.tile([P, P], F32R)
            nc.vector.tensor_copy(identr[:], ident[:])

            # Resident binarized-transposed weights: ST[i, kb, o] = w_bin.T
            st = stpool.tile([P, KT, OUT_F], F32R)

            for ob in range(OUT_F // P):  # 8 blocks of 128 output rows
                wt = wpool.tile([P, IN_F], F32, tag="wt")
                nc.sync.dma_start(wt[:], w[ob * P : (ob + 1) * P, :])
                absw = wpool.tile([P, IN_F], F32, tag="absw")
                alpha = wpool.tile([P, 1], F32, tag="alpha")
                nc.scalar.activation(
                    absw[:], wt[:], mybir.ActivationFunctionType.Abs,
                    accum_out=alpha[:],
                )
                nc.scalar.mul(alpha[:], alpha[:], 1.0 / IN_F)
                # w_bin = sign(w)*alpha via bitwise copysign (alpha > 0):
                # (w & 0x80000000) | bits(alpha) -- one DVE op, no ACT sign
                # pass. (Exact-zero weights would get +/-alpha instead of 0,
                # but fp32 normals are never exactly 0.)
                U32 = mybir.dt.uint32
                sgn = wpool.tile([P, IN_F], F32, tag="sgn")
                nc.vector.tensor_scalar(
                    sgn[:].bitcast(U32),
                    wt[:].bitcast(U32),
                    0x80000000,
                    alpha[:].bitcast(U32),
                    op0=mybir.AluOpType.bitwise_and,
                    op1=mybir.AluOpType.bitwise_or,
                )
                for kb in range(KT):
                    ptile = ptp_pool.tile([P, P], F32, tag="tp")
                    nc.tensor.transpose(
                        ptile[:], sgn[:, kb * P : (kb + 1) * P], ident[:]
                    )
                    nc.vector.tensor_copy(st[:, kb, ob * P : (ob + 1) * P], ptile[:])

            def emit_load_transpose(tt):
                xin = xpool.tile([P, IN_F], F32, tag="xin", name="xin")
                nc.sync.dma_start(xin[:], x[tt * P : (tt + 1) * P, :])
                # round x to fp32r (e8m11) on ACT so the PE transposes run
                # at 1.5 cyc/row instead of fp32's 2 cyc/row
                xr = xpool.tile([P, IN_F], F32R, tag="xr", name="xr")
                nc.scalar.copy(xr[:], xin[:])
                xT = xtpool.tile([P, KT, P], F32R, tag="xT", name="xT")
                for g in range(KT // 4):
                    ptile = ptp_pool.tile([P, 4, P], F32R, tag="tp", name="tp4")
                    for j in range(4):
                        kb = g * 4 + j
                        nc.tensor.transpose(
                            ptile[:, j, :], xr[:, kb * P : (kb + 1) * P],
                            identr[:],
                        )
                    nc.vector.tensor_copy(xT[:, g * 4 : g * 4 + 4, :], ptile[:])
                return xT

            def emit_mms(tt, xT):
                psum = pmm_pool.tile([P, OUT_F], F32, tag="acc", name="acc")
                for kb in range(KT):
                    for n in range(NT):
                        nc.tensor.matmul(
                            psum[:, n * NFREE : (n + 1) * NFREE],
                            xT[:, kb, :],
                            st[:, kb, n * NFREE : (n + 1) * NFREE],
                            start=(kb == 0),
                            stop=(kb == KT - 1),
                        )
                ot = opool.tile([P, OUT_F], F32, tag="ot", name="ot")
                nc.scalar.copy(ot[:], psum[:])
                nc.scalar.dma_start(out[tt * P : (tt + 1) * P, :], ot[:])

            # software pipeline: tile tt+1's transposes are emitted (and so
            # scheduled on the in-order PE) before tile tt's matmuls, giving
            # the DVE drain a full MM-phase of slack.
            pending = None
            for _rep in range(reps):
                for tt in range(T_TILES):
                    xT = emit_load_transpose(tt)
                    if pending is not None:
                        emit_mms(*pending)
                    pending = (tt, xT)
            if pending is not None:
                emit_mms(*pending)

    nc.finalize()
    return nc


_NC_CACHE: dict = {}


def _get_nc(reps: int = 1):
    if reps not in _NC_CACHE:
        _NC_CACHE[reps] = build_nc(reps)
    return _NC_CACHE[reps]


def _make_runner(nc, n_cores=N_CORES):
    """Cached-jit SPMD runner on the bass2jax/PJRT path (axon-compatible):
    one jax.jit per Bass module, reused across kernel() calls."""
    import jax
    from jax.experimental.shard_map import shard_map
    from jax.sharding import Mesh, PartitionSpec
    from concourse.bass2jax import (
        _bass_exec_p,
        install_neuronx_cc_hook,
        partition_id_tensor,
    )

    install_neuronx_cc_hook()
    partition_name = nc.partition_id_tensor.name if nc.partition_id_tensor else None

    in_names, out_names, out_avals, out_shapes = [], [], [], []
    for alloc in nc.m.functions[0].allocations:
        if not isinstance(alloc, mybir.MemoryLocationSet):
            continue
        name = alloc.memorylocations[0].name
        if alloc.kind == "ExternalInput":
            if name != partition_name:
                in_names.append(name)
        elif alloc.kind == "ExternalOutput":
            shape = tuple(alloc.tensor_shape)
            dtype = mybir.dt.np(alloc.dtype)
            out_names.append(name)
            out_avals.append(jax.core.ShapedArray(shape, dtype))
            out_shapes.append((shape, dtype))
    n_params = len(in_names)
    all_in_names = list(in_names) + list(out_names)
    if partition_name is not None:
        all_in_names.append(partition_name)

    def _body(*args):
        operands = list(args)
        if partition_name is not None:
            operands.append(partition_id_tensor())
        outs = _bass_exec_p.bind(
            *operands,
            out_avals=tuple(out_avals),
            in_names=tuple(all_in_names),
            out_names=tuple(out_names),
            lowering_input_output_aliases=(),
            sim_require_finite=True,
            sim_require_nnan=True,
            nc=nc,
        )
        return tuple(outs)

    devices = jax.devices()[:n_cores]
    mesh = Mesh(np.asarray(devices), ("core",))
    nspec = (PartitionSpec("core"),)
    sharded = jax.jit(
        shard_map(
            _body,
            mesh=mesh,
            in_specs=nspec * (n_params + len(out_names)),
            out_specs=nspec * len(out_names),
            check_rep=False,
        ),
        keep_unused=True,
    )

    def run(arrays_by_name):
        concat_in = [arrays_by_name[nm] for nm in in_names]
        zeros = [
            np.zeros((n_cores * s[0], *s[1:]), dt) for (s, dt) in out_shapes
        ]
        out_arrs = sharded(*concat_in, *zeros)
        jax.block_until_ready(out_arrs)
        return {nm: np.asarray(out_arrs[i]) for i, nm in enumerate(out_names)}

    return run


_RUNNER_CACHE: dict = {}


def _get_runner(reps: int = 1):
    if reps not in _RUNNER_CACHE:
        _RUNNER_CACHE[reps] = _make_runner(_get_nc(reps))
    return _RUNNER_CACHE[reps]


def kernel(x: np.ndarray, weight: np.ndarray) -> np.ndarray:
    x = np.ascontiguousarray(np.asarray(x, dtype=np.float32))
    weight = np.ascontiguousarray(np.asarray(weight, dtype=np.float32))
    assert x.shape == (TOKENS, IN_F) and weight.shape == (OUT_F, IN_F)

    run = _get_runner()
    # shard_map splits axis 0 across the 8 cores: x is already the
    # token-concat of the shards; w must be stacked 8x (replication).
    outs = run({"x": x, "w": np.concatenate([weight] * N_CORES, axis=0)})
    return outs["out"]  # [TOKENS, OUT_F] -- concat of per-core shards



# revision 8
# speedup vs baseline: 9.7639x; 9.7639x over previous
"""BinaryLinear (65536x1024 @ binarized 1024x1024) on 8 TRN2 NeuronCores.

out = x @ (sign(w) * mean(|w|, axis=1)).T

Strategy (data-parallel per sharding hint): shard x along tokens (8192
rows/core), replicate w.

v2 design -- all transposes on the DMA xbar, hybrid bf16/fp8 matmul:
  - weights once per core: alpha = mean|w| per output row (ACT Abs with
    fused accumulate); sign matrix as +/-1 bf16 via one DVE bitwise
    copysign; transposed into resident ST via dma_start_transpose (xbar
    16x128 tiles, 2-byte dtype); last K8 contraction planes additionally
    converted to +/-1 fp8e4m3 (exact). alpha itself is transposed to a
    [1,1024] row on the PE and broadcast to a [128,1024] SBUF tile
    (gpsimd partition_broadcast) in full fp32.
  - per 128-token tile: DMA in, ACT converts to bf16, ONE
    dma_start_transpose yields xT[i, plane, t] for all 8 planes, ACT
    converts the K8 fp8 planes; PE runs (8-K8) bf16 matmuls x2 n-slices
    at 1 cyc/row plus K8/2 fp8 DoubleRow matmuls (2 contraction planes
    per instruction) accumulating into one fp32 PSUM tile; Pool engine
    multiplies PSUM by the broadcast alpha (per-output-column, exact
    fp32) while draining to SBUF; store DMA on the ACT ring.
  - x loads ride the SP ring, stores the ACT ring, xbar transposes
    alternate between the two; the PE stream is pure back-to-back
    matmuls (software pipeline, lookahead 2).

Accuracy: weights are exactly +/-1 in both bf16 and fp8; alpha is
applied in fp32, so the only error is x-quantization: bf16 planes
~1e-4, fp8 planes dominate. K8=2 gives max-rel ~1.5e-2 (gate 2e-2),
K8=0 (pure bf16) ~1.6e-3.
"""

import sys

for _p in ("/opt/trn_rl_repo", "/root/.axon_site/_ro/trn_rl_repo"):
    if _p not in sys.path:
        sys.path.insert(0, _p)

import numpy as np

import concourse.mybir as mybir
import concourse.tile as tile
from concourse import bacc
from concourse.masks import make_identity

TOKENS, IN_F, OUT_F = 65536, 1024, 1024
N_CORES = 8
T_PER_CORE = TOKENS // N_CORES  # 8192
P = 128
T_TILES = T_PER_CORE // P  # 64
KT = IN_F // P  # 8 contraction planes
K8 = 2  # fp8 DoubleRow planes (last K8 of KT); must be even
NFREE = 512  # bf16 matmul moving width
N8 = 256  # DoubleRow output width (moving 2x256=512)

F32 = mybir.dt.float32
BF16 = mybir.dt.bfloat16
FP8 = mybir.dt.float8e4
U16 = mybir.dt.uint16

import os as _os

# debug/tuning knobs (env): transpose ring assignment + sw pipeline depth
TMODE = _os.environ.get("K_TMODE", "sync")  # sync | scalar
LOOKAHEAD = int(_os.environ.get("K_LOOKAHEAD", "2"))


def build_nc(reps: int = 1, t_tiles: int = T_TILES, k8: int = K8):
    kb = KT - k8
    assert k8 % 2 == 0
    nc = bacc.Bacc()
    x = nc.declare_dram_parameter("x", [t_tiles * P, IN_F], F32, isOutput=False)
    w = nc.declare_dram_parameter("w", [OUT_F, IN_F], F32, isOutput=False)
    out = nc.declare_dram_parameter("out", [t_tiles * P, OUT_F], F32, isOutput=True)

    with tile.TileContext(nc) as tc:
        with (
            tc.tile_pool(name="const", bufs=1) as cpool,
            tc.tile_pool(name="st", bufs=1) as stpool,
            tc.tile_pool(name="wproc", bufs=2) as wpool,
            tc.tile_pool(name="xin", bufs=4) as xpool,
            tc.tile_pool(name="xb", bufs=4) as xbpool,
            tc.tile_pool(name="xt", bufs=4) as xtpool,
            tc.tile_pool(name="outp", bufs=3) as opool,
            tc.tile_pool(name="pmm", bufs=3, space="PSUM") as pmm_pool,
            tc.tile_pool(name="pa", bufs=2, space="PSUM") as pa_pool,
        ):
            ident = cpool.tile([P, P], F32)
            make_identity(nc, ident[:])
            ones1 = cpool.tile([1, P], F32)
            nc.vector.memset(ones1[:], 1.0)

            # Resident transposed sign weights: stb[i, kb, o] = sign(w[o, i])
            stb = stpool.tile([P, KT, OUT_F], BF16)
            st8 = stpool.tile([P, max(k8, 2), OUT_F], FP8, name="st8")
            alpha_bc = stpool.tile([P, OUT_F], F32)
            arow = cpool.tile([1, OUT_F], F32)

            for ob in range(OUT_F // P):
                wt = wpool.tile([P, IN_F], F32, tag="wt")
                nc.sync.dma_start(wt[:], w[ob * P : (ob + 1) * P, :])
                absw = wpool.tile([P, IN_F], F32, tag="absw")
                alpha = wpool.tile([P, 1], F32, tag="alpha")
                nc.scalar.activation(
                    absw[:], wt[:], mybir.ActivationFunctionType.Abs,
                    accum_out=alpha[:],
                )
                wb = wpool.tile([P, IN_F], BF16, tag="wb")
                nc.scalar.copy(wb[:], wt[:])
                # sign as +/-1 bf16: (w & 0x8000) | bits(1.0f_bf16)
                sgn = wpool.tile([P, IN_F], BF16, tag="sgn")
                nc.vector.tensor_scalar(
                    sgn[:].bitcast(U16),
                    wb[:].bitcast(U16),
                    0x8000,
                    0x3F80,
                    op0=mybir.AluOpType.bitwise_and,
                    op1=mybir.AluOpType.bitwise_or,
                )
                # xbar transpose needs a CONTIGUOUS SBUF destination
                # (strided dest corrupts elements on HW); stage then copy.
                stg = wpool.tile([P, KT, P], BF16, tag="stg")
                nc.sync.dma_start_transpose(stg[:], sgn[:])
                nc.vector.tensor_copy(stb[:, :, ob * P : (ob + 1) * P], stg[:])
                # alpha row: PE-transpose the [128,1] column to [1,128]
                pt = pa_pool.tile([1, P], F32, tag="pt")
                nc.tensor.transpose(pt[:], alpha[:], ident[:])
                nc.scalar.mul(arow[0:1, ob * P : (ob + 1) * P], pt[:], 1.0 / IN_F)

            # broadcast arow [1,1024] -> alpha_bc [128,1024] via PE
            # (ones[1,128].T @ arow-slice), fp32 exact; gpsimd
            # partition_broadcast races with the main loop on HW.
            pbc = pmm_pool.tile([P, OUT_F], F32, tag="acc", name="pbc")
            for n in range(OUT_F // NFREE):
                nc.tensor.matmul(
                    pbc[:, n * NFREE : (n + 1) * NFREE],
                    ones1[:],
                    arow[0:1, n * NFREE : (n + 1) * NFREE],
                    start=True,
                    stop=True,
                )
            nc.scalar.copy(alpha_bc[:], pbc[:])
            if k8:
                nc.scalar.copy(st8[:], stb[:, kb:KT, :])

            def emit_producers(tt):
                xin = xpool.tile([P, IN_F], F32, tag="xin", name="xin")
                nc.sync.dma_start(xin[:], x[tt * P : (tt + 1) * P, :])
                xb = xbpool.tile([P, IN_F], BF16, tag="xb", name="xb")
                nc.scalar.copy(xb[:], xin[:])
                xT = xtpool.tile([P, KT, P], BF16, tag="xT", name="xT")
                # all xbar transposes on ONE ring: concurrent transposes
                # on both rings corrupt each other on HW.
                getattr(nc, TMODE).dma_start_transpose(xT[:], xb[:])
                if k8:
                    xT8 = xtpool.tile([P, k8, P], FP8, tag="xT8", name="xT8")
                    nc.scalar.copy(xT8[:], xT[:, kb:KT, :])
                else:
                    xT8 = None
                return xT, xT8

            def emit_consumers(tt, xT, xT8):
                psum = pmm_pool.tile([P, OUT_F], F32, tag="acc", name="acc")
                for k in range(kb):
                    for n in range(OUT_F // NFREE):
                        nc.tensor.matmul(
                            psum[:, n * NFREE : (n + 1) * NFREE],
                            xT[:, k, :],
                            stb[:, k, n * NFREE : (n + 1) * NFREE],
                            start=(k == 0),
                            stop=(k8 == 0 and k == kb - 1),
                        )
                for g in range(k8 // 2):
                    for n in range(OUT_F // N8):
                        nc.tensor.matmul(
                            psum[:, n * N8 : (n + 1) * N8],
                            xT8[:, 2 * g : 2 * g + 2, :],
                            st8[:, 2 * g : 2 * g + 2, n * N8 : (n + 1) * N8],
                            start=(kb == 0 and g == 0),
                            stop=(g == k8 // 2 - 1),
                            perf_mode=mybir.MatmulPerfMode.DoubleRow,
                        )
                outt = opool.tile([P, OUT_F], F32, tag="outt", name="outt")
                nc.vector.tensor_tensor(
                    outt[:], psum[:], alpha_bc[:], op=mybir.AluOpType.mult
                )
                nc.scalar.dma_start(out[tt * P : (tt + 1) * P, :], outt[:])

            # software pipeline: producers run LOOKAHEAD tiles ahead so the
            # PE stream is back-to-back matmuls.
            pending = []
            for _rep in range(reps):
                for tt in range(t_tiles):
                    pending.append((tt, *emit_producers(tt)))
                    if len(pending) > LOOKAHEAD:
                        emit_consumers(*pending.pop(0))
            for args in pending:
                emit_consumers(*args)

    nc.finalize()
    return nc


_NC_CACHE: dict = {}


def _get_nc(reps: int = 1, t_tiles: int = T_TILES, k8: int = K8):
    key = (reps, t_tiles, k8)
    if key not in _NC_CACHE:
        _NC_CACHE[key] = build_nc(reps, t_tiles, k8)
    return _NC_CACHE[key]


def _make_runner(nc, n_cores=N_CORES):
    """Cached-jit SPMD runner on the bass2jax/PJRT path (axon-compatible):
    one jax.jit per Bass module, reused across kernel() calls."""
    import jax
    from jax.experimental.shard_map import shard_map
    from jax.sharding import Mesh, PartitionSpec
    from concourse.bass2jax import (
        _bass_exec_p,
        install_neuronx_cc_hook,
        partition_id_tensor,
    )

    install_neuronx_cc_hook()
    partition_name = nc.partition_id_tensor.name if nc.partition_id_tensor else None

    in_names, out_names, out_avals, out_shapes = [], [], [], []
    for alloc in nc.m.functions[0].allocations:
        if not isinstance(alloc, mybir.MemoryLocationSet):
            continue
        name = alloc.memorylocations[0].name
        if alloc.kind == "ExternalInput":
            if name != partition_name:
                in_names.append(name)
        elif alloc.kind == "ExternalOutput":
            shape = tuple(alloc.tensor_shape)
            dtype = mybir.dt.np(alloc.dtype)
            out_names.append(name)
            out_avals.append(jax.core.ShapedArray(shape, dtype))
            out_shapes.append((shape, dtype))
    n_params = len(in_names)
    all_in_names = list(in_names) + list(out_names)
    if partition_name is not None:
        all_in_names.append(partition_name)

    def _body(*args):
        operands = list(args)
        if partition_name is not None:
            operands.append(partition_id_tensor())
        outs = _bass_exec_p.bind(
            *operands,
            out_avals=tuple(out_avals),
            in_names=tuple(all_in_names),
            out_names=tuple(out_names),
            lowering_input_output_aliases=(),
            sim_require_finite=True,
            sim_require_nnan=True,
            nc=nc,
        )
        return tuple(outs)

    devices = jax.devices()[:n_cores]
    mesh = Mesh(np.asarray(devices), ("core",))
    nspec = (PartitionSpec("core"),)
    sharded = jax.jit(
        shard_map(
            _body,
            mesh=mesh,
            in_specs=nspec * (n_params + len(out_names)),
            out_specs=nspec * len(out_names),
            check_rep=False,
        ),
        keep_unused=True,
    )

    def run(arrays_by_name):
        import jax as _jax

        concat_in = [arrays_by_name[nm] for nm in in_names]
        zeros = [
            np.zeros((n_cores * s[0], *s[1:]), dt) for (s, dt) in out_shapes
        ]
        out_arrs = sharded(*concat_in, *zeros)
        _jax.block_until_ready(out_arrs)
        return {nm: np.asarray(out_arrs[i]) for i, nm in enumerate(out_names)}

    return run


_RUNNER_CACHE: dict = {}


def _get_runner(reps: int = 1, t_tiles: int = T_TILES, k8: int = K8):
    key = (reps, t_tiles, k8)
    if key not in _RUNNER_CACHE:
        _RUNNER_CACHE[key] = _make_runner(_get_nc(reps, t_tiles, k8))
    return _RUNNER_CACHE[key]


def kernel(x: np.ndarray, weight: np.ndarray) -> np.ndarray:
    x = np.ascontiguousarray(np.asarray(x, dtype=np.float32))
    weight = np.ascontiguousarray(np.asarray(weight, dtype=np.float32))
    assert x.shape == (TOKENS, IN_F) and weight.shape == (OUT_F, IN_F)

    run = _get_runner()
    # shard_map splits axis 0 across the 8 cores: x is already the
    # token-concat of the shards; w must be stacked 8x (replication).
    outs = run({"x": x, "w": np.concatenate([weight] * N_CORES, axis=0)})
    return outs["out"]  # [TOKENS, OUT_F] -- concat of per-core shards
